# revision 12
# baseline (speedup 1.0000x reference)
"""LinearAttention Trainium2 kernel (8 NeuronCores, sequence-sharded).

Reference computation (per batch b):
    qkv = x @ W_qkv; q,k,v split; per-head: softmax(q, dim=dh),
    softmax(k, dim=seq); ctx = k^T v; out = q_sm @ ctx; y = out @ W_out + b.

Device dataflow per core (sequence shard of 1024 rows x 2 batches):
  phase 1: qkv = xT.T @ Wq (bf16 matmuls, f32 PSUM), natural layout
           exp_q via one ACT instr, per-head row sums via DVE tensor_reduce,
           q_sm = exp_q * rq (8 DVE muls), exp_k + v copy on ACT,
           q_sm transposed into qsmT via PE transposes
  phase 2: per-head ctx and Z colsums accumulated in PSUM chains across the
           8 tiles of a batch; flushed to bf16 and DMAed per batch
  phase 3: 66KB bf16 AllReduce of [ctxT | Z] over the 8 cores (per batch)
  phase 4: M_t = ctx_t @ W_out_t with 1/Z folded in on the PSUM->SBUF copy
  phase 5: y = sum_t qsmT_t.T @ M_t; y stored bf16
Host: shards/transposes/casts x, gathers per-core y shards, adds b_out.
"""
import numpy as np
import ml_dtypes
from contextlib import ExitStack

import concourse.bass as bass
import concourse.mybir as mybir
import concourse.tile as tile
from concourse import bacc
from concourse.bass_utils import run_bass_kernel_spmd
from concourse.masks import make_identity

bf16 = ml_dtypes.bfloat16
F32 = mybir.dt.float32
BF = mybir.dt.bfloat16
EXP = mybir.ActivationFunctionType.Exp
COPY = mybir.ActivationFunctionType.Copy

B, N, D = 2, 8192, 1024
H, DH, INNER = 8, 64, 512
NCORES = 8
NL = N // NCORES            # 1024 seq rows per batch per core
SEQ = B * NL                # 2048 rows per core
NT_B = NL // 128            # 8 seq tiles per batch


def _body(tc, xT, wq, wo, y):
    nc = tc.nc
    with ExitStack() as ctx:
        const = ctx.enter_context(tc.tile_pool(name="const", bufs=1))
        dram = ctx.enter_context(tc.tile_pool(name="dram", bufs=1, space="DRAM"))

        ident = const.tile([128, 128], BF)
        make_identity(nc, ident)
        ones128 = const.tile([128, 1], BF)
        nc.vector.memset(ones128, 1.0)

        # weights: wq chunks loaded just-in-time behind the first x tile
        wq_sb = const.tile([128, 8, 3 * INNER], BF)
        wo_sb = const.tile([64, 8, D], BF)  # head-major W_out rows

        qsmT = const.tile([128, 4, SEQ], BF)  # persistent q_sm^T
        xT_r = xT[:].rearrange("(c p) s -> p c s", p=128)  # [128, 8, 2048]

        red = []  # allreduced [ctxT | Z] per batch, bf16
        with ExitStack() as p12:
            xt_pool = p12.enter_context(tc.tile_pool(name="xt", bufs=3))
            work = p12.enter_context(tc.tile_pool(name="work", bufs=3))
            small = p12.enter_context(tc.tile_pool(name="small", bufs=4))
            czs_pool = p12.enter_context(tc.tile_pool(name="czs", bufs=2))
            qk_psum = p12.enter_context(tc.tile_pool(name="qk_ps", bufs=2, space="PSUM"))
            v_psum = p12.enter_context(tc.tile_pool(name="v_ps", bufs=1, space="PSUM"))
            ctx_psum = p12.enter_context(tc.tile_pool(name="ctx_ps", bufs=1, space="PSUM"))
            tr_psum = p12.enter_context(tc.tile_pool(name="tr_ps", bufs=2, space="PSUM"))

            for b in range(B):
                cz_acc = czs_pool.tile([65, 512], F32, tag="cz_acc")
                nc.vector.memset(cz_acc, 0.0)
                for mb in range(NT_B):
                    m = b * NT_B + mb
                    xt = xt_pool.tile([128, 8, 128], BF, tag="xt")
                    nc.sync.dma_start(out=xt, in_=xT_r[:, :, m * 128:(m + 1) * 128])
                    if m == 0:
                        for kk in range(8):
                            nc.sync.dma_start(
                                out=wq_sb[:, kk, :],
                                in_=wq[128 * kk:128 * (kk + 1), :])

                    qkv_ps = qk_psum.tile([128, 1024], F32, tag="qk")
                    v_ps = v_psum.tile([128, 512], F32, tag="vp")
                    for kk in range(8):
                        for nb in range(2):
                            nc.tensor.matmul(
                                qkv_ps[:, nb * 512:(nb + 1) * 512],
                                lhsT=xt[:, kk, :],
                                rhs=wq_sb[:, kk, nb * 512:(nb + 1) * 512],
                                start=(kk == 0), stop=(kk == 7))
                    for kk in range(8):
                        nc.tensor.matmul(
                            v_ps, lhsT=xt[:, kk, :],
                            rhs=wq_sb[:, kk, 1024:1536],
                            start=(kk == 0), stop=(kk == 7))

                    # q softmax (over dh) in natural layout
                    expq = work.tile([128, 8, DH], BF, tag="expq")
                    nc.scalar.activation(out=expq, in_=qkv_ps[:, 0:512], func=EXP)
                    qsum = small.tile([128, 8], F32, tag="qsum")
                    nc.vector.tensor_reduce(
                        out=qsum, in_=expq, axis=mybir.AxisListType.X,
                        op=mybir.AluOpType.add)
                    rq = small.tile([128, 8], F32, tag="rq")
                    nc.vector.reciprocal(rq, qsum)
                    qsm = work.tile([128, 8, DH], BF, tag="qsm")
                    for h in range(H):
                        eng = nc.vector if h % 2 == 0 else nc.gpsimd
                        eng.tensor_scalar_mul(
                            qsm[:, h, :], expq[:, h, :], rq[:, h:h + 1])

                    expk = work.tile([128, INNER], BF, tag="expk")
                    nc.scalar.activation(out=expk, in_=qkv_ps[:, 512:1024], func=EXP)
                    # v with a ones column appended per head: ctx matmul row 64
                    # then yields that head's exp_k colsums (Z) for free
                    vones = work.tile([128, 8, 65], BF, tag="vones")
                    nc.scalar.activation(
                        out=vones[:, :, 0:64], in_=v_ps[:, :].rearrange(
                            "p (h e) -> p h e", h=H), func=COPY)
                    nc.vector.memset(vones[:, :, 64:65], 1.0)

                    # ctx+Z: out [65, 64] per head at cols 64h
                    cz = ctx_psum.tile([65, 512], F32, tag="cz")
                    for h in range(H):
                        nc.tensor.matmul(
                            cz[0:65, 64 * h:64 * (h + 1)],
                            lhsT=vones[:, h, :],
                            rhs=expk[:, 64 * h:64 * (h + 1)],
                            start=True, stop=True)
                    nc.vector.tensor_add(cz_acc, cz_acc, cz)

                    # transpose q_sm into qsmT
                    for t4 in range(4):
                        trp = tr_psum.tile([128, 128], BF, tag="tr")
                        nc.tensor.transpose(
                            trp, qsm[:, 2 * t4:2 * t4 + 2, :], ident)
                        if t4 % 2 == 0:
                            nc.vector.tensor_copy(
                                out=qsmT[:, t4, m * 128:(m + 1) * 128], in_=trp)
                        else:
                            nc.scalar.activation(
                                out=qsmT[:, t4, m * 128:(m + 1) * 128],
                                in_=trp, func=COPY)

                cz_sb = czs_pool.tile([65, 512], BF, tag="cz_sb")
                nc.vector.tensor_copy(out=cz_sb, in_=cz_acc)
                part_b = dram.tile([65, 512], BF, tag=f"part{b}")
                red_b = dram.tile([65, 512], BF, tag=f"red{b}")
                nc.sync.dma_start(out=part_b, in_=cz_sb)
                nc.gpsimd.collective_compute(
                    "AllReduce", mybir.AluOpType.add,
                    replica_groups=[list(range(NCORES))],
                    ins=[part_b.opt()], outs=[red_b.opt()])
                red.append(red_b)

        # wo loaded behind phase-1 traffic; only needed for phase 4
        for h in range(8):
            nc.sync.dma_start(out=wo_sb[:, h, :], in_=wo[64 * h:64 * (h + 1), :])

        with ExitStack() as p45:
            work2 = p45.enter_context(tc.tile_pool(name="work2", bufs=2))
            small2 = p45.enter_context(tc.tile_pool(name="small2", bufs=2))
            ysb_pool = p45.enter_context(tc.tile_pool(name="ysb", bufs=4))
            m_psum = p45.enter_context(tc.tile_pool(name="m_ps", bufs=2, space="PSUM"))
            y_psum = p45.enter_context(tc.tile_pool(name="y_ps", bufs=4, space="PSUM"))

            for b in range(B):
                red_sb = work2.tile([65, 512], BF, tag="red")
                nc.sync.dma_start(out=red_sb, in_=red[b])
                # Z row -> per-partition [128, 4] via 4 tiny transposes
                ztp = m_psum.tile([128, 4, 2], BF, tag="ztp")
                for j in range(4):
                    nc.tensor.transpose(
                        ztp[:, j, 0:1],
                        red_sb[64:65, 128 * j:128 * (j + 1)],
                        ones128[64:65, :])
                rz = small2.tile([128, 4], F32, tag="rz")
                nc.vector.reciprocal(rz, ztp[:, :, 0])

                # M_t = ctx_t @ Wout_t with 1/Z folded in on the copy out
                # head h = 2t + r: ctx_h = red_sb[0:64, 64h:64h+64]
                m_sb = work2.tile([128, 4, D], BF, tag="msb")
                for t in range(4):
                    for cb in range(2):
                        mp = m_psum.tile([128, 512], F32, tag="mp")
                        for r in range(2):
                            h = 2 * t + r
                            nc.tensor.matmul(
                                mp[64 * r:64 * (r + 1), :],
                                lhsT=red_sb[0:64, 64 * h:64 * (h + 1)],
                                rhs=wo_sb[:, h, cb * 512:(cb + 1) * 512],
                                start=True, stop=True)
                        if cb == 0:
                            nc.scalar.activation(
                                out=m_sb[:, t, cb * 512:(cb + 1) * 512],
                                in_=mp, func=COPY, scale=rz[:, t:t + 1])
                        else:
                            nc.vector.tensor_scalar_mul(
                                m_sb[:, t, cb * 512:(cb + 1) * 512], mp,
                                rz[:, t:t + 1])

                # y = sum_t qsmT_t^T @ M_t
                for mi in range(NT_B):
                    for cb in range(2):
                        yp = y_psum.tile([128, 512], F32, tag="yp")
                        for t in range(4):
                            nc.tensor.matmul(
                                yp, lhsT=qsmT[:, t, b * NL + mi * 128:
                                              b * NL + (mi + 1) * 128],
                                rhs=m_sb[:, t, cb * 512:(cb + 1) * 512],
                                start=(t == 0), stop=(t == 3))
                        ysb = ysb_pool.tile([128, 512], BF, tag="ysb")
                        if cb == 0:
                            nc.scalar.activation(out=ysb, in_=yp, func=COPY)
                        else:
                            nc.vector.tensor_copy(out=ysb, in_=yp)
                        nc.sync.dma_start(
                            out=y[b * NL + mi * 128: b * NL + (mi + 1) * 128,
                                  cb * 512:(cb + 1) * 512],
                            in_=ysb)


_COMPILED = None


def _build():
    global _COMPILED
    if _COMPILED is None:
        nc = bacc.Bacc("TRN2", target_bir_lowering=False, debug=False,
                       num_devices=NCORES)
        xT = nc.declare_dram_parameter("xT", [D, SEQ], BF, isOutput=False)
        wq = nc.declare_dram_parameter("wq", [D, 3 * INNER], BF, isOutput=False)
        wo = nc.declare_dram_parameter("wo", [INNER, D], BF, isOutput=False)
        y = nc.declare_dram_parameter("y", [SEQ, D], BF, isOutput=True)
        with tile.TileContext(nc) as tc:
            _body(tc, xT, wq, wo, y)
        nc.compile()
        _COMPILED = nc
    return _COMPILED


def _make_in_maps(x, W_qkv, W_out):
    wq_bf = np.ascontiguousarray(W_qkv).astype(bf16)
    wo_bf = np.ascontiguousarray(W_out).astype(bf16)
    in_maps = []
    for c in range(NCORES):
        rows = slice(c * NL, (c + 1) * NL)
        xs = np.concatenate([x[0, rows], x[1, rows]], axis=0)  # [2048, 1024]
        xT_bf = np.ascontiguousarray(xs.T).astype(bf16)        # [1024, 2048]
        in_maps.append({"xT": xT_bf, "wq": wq_bf, "wo": wo_bf})
    return in_maps


def _run(x, W_qkv, W_out, b_out, trace=False, **spmd_kwargs):
    nc = _build()
    in_maps = _make_in_maps(x, W_qkv, W_out)
    res = run_bass_kernel_spmd(nc, in_maps, list(range(NCORES)),
                               trace=trace, **spmd_kwargs)
    out = np.empty((B, N, D), np.float32)
    for c in range(NCORES):
        yc = np.asarray(res.results[c]["y"]).astype(np.float32)
        rows = slice(c * NL, (c + 1) * NL)
        out[0, rows] = yc[:NL]
        out[1, rows] = yc[NL:]
    out += np.asarray(b_out, np.float32)[None, None, :]
    return out, res


def kernel(x, W_qkv, W_out, b_out):
    x = np.asarray(x, np.float32)
    out, _ = _run(x, np.asarray(W_qkv, np.float32),
                  np.asarray(W_out, np.float32),
                  np.asarray(b_out, np.float32))
    return out


# revision 13
# speedup vs baseline: 1.2419x; 1.2419x over previous
"""LinearAttention Trainium2 kernel (8 NeuronCores, sequence-sharded).

Reference computation (per batch b):
    qkv = x @ W_qkv; q,k,v split; per-head: softmax(q, dim=dh),
    softmax(k, dim=seq); ctx = k^T v; out = q_sm @ ctx; y = out @ W_out + b.

Device dataflow per core (sequence shard of 1024 rows x 2 batches):
  phase 1: qkv = xT.T @ Wq (bf16 matmuls, f32 PSUM), natural layout
           exp_q via one ACT instr, per-head row sums via DVE tensor_reduce,
           q_sm = exp_q * rq (8 DVE muls), exp_k + v copy on ACT,
           q_sm transposed into qsmT via PE transposes
  phase 2: per-head ctx and Z colsums accumulated in PSUM chains across the
           8 tiles of a batch; flushed to bf16 and DMAed per batch
  phase 3: 66KB bf16 AllReduce of [ctxT | Z] over the 8 cores (per batch)
  phase 4: M_t = ctx_t @ W_out_t with 1/Z folded in on the PSUM->SBUF copy
  phase 5: y = sum_t qsmT_t.T @ M_t; y stored bf16
Host: shards/transposes/casts x, gathers per-core y shards, adds b_out.
"""
import numpy as np
import ml_dtypes
from contextlib import ExitStack

import concourse.bass as bass
import concourse.mybir as mybir
import concourse.tile as tile
from concourse import bacc
from concourse.bass_utils import run_bass_kernel_spmd
from concourse.masks import make_identity

bf16 = ml_dtypes.bfloat16
F32 = mybir.dt.float32
BF = mybir.dt.bfloat16
EXP = mybir.ActivationFunctionType.Exp
COPY = mybir.ActivationFunctionType.Copy

B, N, D = 2, 8192, 1024
H, DH, INNER = 8, 64, 512
NCORES = 8
NL = N // NCORES            # 1024 seq rows per batch per core
SEQ = B * NL                # 2048 rows per core
NT_B = NL // 128            # 8 seq tiles per batch


def _body(tc, xT, wq, wo, y):
    nc = tc.nc
    with ExitStack() as ctx:
        const = ctx.enter_context(tc.tile_pool(name="const", bufs=1))
        dram = ctx.enter_context(tc.tile_pool(name="dram", bufs=1, space="DRAM"))

        ident = const.tile([128, 128], BF)
        make_identity(nc, ident)
        ones128 = const.tile([128, 1], BF)
        nc.vector.memset(ones128, 1.0)

        # weights: wq chunks loaded just-in-time behind the first x tile
        wq_sb = const.tile([128, 8, 3 * INNER], BF)
        wo_sb = const.tile([64, 8, D], BF)  # head-major W_out rows

        qsmT = const.tile([128, 4, SEQ], BF)  # persistent q_sm^T
        xT_r = xT[:].rearrange("(c p) s -> p c s", p=128)  # [128, 8, 2048]

        red = []  # allreduced [ctxT | Z] per batch, bf16
        with ExitStack() as p12:
            xt_pool = p12.enter_context(tc.tile_pool(name="xt", bufs=3))
            work = p12.enter_context(tc.tile_pool(name="work", bufs=3))
            small = p12.enter_context(tc.tile_pool(name="small", bufs=4))
            czs_pool = p12.enter_context(tc.tile_pool(name="czs", bufs=2))
            qk_psum = p12.enter_context(tc.tile_pool(name="qk_ps", bufs=2, space="PSUM"))
            v_psum = p12.enter_context(tc.tile_pool(name="v_ps", bufs=1, space="PSUM"))
            ctx_psum = p12.enter_context(tc.tile_pool(name="ctx_ps", bufs=1, space="PSUM"))
            tr_psum = p12.enter_context(tc.tile_pool(name="tr_ps", bufs=2, space="PSUM"))

            for b in range(B):
                cz_acc = czs_pool.tile([65, 512], F32, tag="cz_acc")
                nc.vector.memset(cz_acc, 0.0)
                for mb in range(NT_B):
                    m = b * NT_B + mb
                    xt = xt_pool.tile([128, 8, 128], BF, tag="xt")
                    nc.sync.dma_start(out=xt, in_=xT_r[:, :, m * 128:(m + 1) * 128])
                    if m == 0:
                        for kk in range(8):
                            nc.sync.dma_start(
                                out=wq_sb[:, kk, :],
                                in_=wq[128 * kk:128 * (kk + 1), :])

                    qkv_ps = qk_psum.tile([128, 1024], F32, tag="qk")
                    v_ps = v_psum.tile([128, 512], F32, tag="vp")
                    for kk in range(8):
                        for nb in range(2):
                            nc.tensor.matmul(
                                qkv_ps[:, nb * 512:(nb + 1) * 512],
                                lhsT=xt[:, kk, :],
                                rhs=wq_sb[:, kk, nb * 512:(nb + 1) * 512],
                                start=(kk == 0), stop=(kk == 7))
                    for kk in range(8):
                        nc.tensor.matmul(
                            v_ps, lhsT=xt[:, kk, :],
                            rhs=wq_sb[:, kk, 1024:1536],
                            start=(kk == 0), stop=(kk == 7))

                    # q softmax (over dh) in natural layout
                    expq = work.tile([128, 8, DH], BF, tag="expq")
                    nc.scalar.activation(out=expq, in_=qkv_ps[:, 0:512], func=EXP)
                    qsum = small.tile([128, 8], F32, tag="qsum")
                    nc.vector.tensor_reduce(
                        out=qsum, in_=expq, axis=mybir.AxisListType.X,
                        op=mybir.AluOpType.add)
                    rq = small.tile([128, 8], F32, tag="rq")
                    nc.vector.reciprocal(rq, qsum)
                    qsm = work.tile([128, 8, DH], BF, tag="qsm")
                    for h in range(H):
                        nc.vector.tensor_scalar_mul(
                            qsm[:, h, :], expq[:, h, :], rq[:, h:h + 1])

                    expk = work.tile([128, INNER], BF, tag="expk")
                    nc.scalar.activation(out=expk, in_=qkv_ps[:, 512:1024], func=EXP)
                    # v with a ones column appended per head: ctx matmul row 64
                    # then yields that head's exp_k colsums (Z) for free
                    vones = work.tile([128, 8, 65], BF, tag="vones")
                    nc.scalar.activation(
                        out=vones[:, :, 0:64], in_=v_ps[:, :].rearrange(
                            "p (h e) -> p h e", h=H), func=COPY)
                    nc.vector.memset(vones[:, :, 64:65], 1.0)

                    # ctx+Z: out [65, 64] per head at cols 64h
                    cz = ctx_psum.tile([65, 512], F32, tag="cz")
                    for h in range(H):
                        nc.tensor.matmul(
                            cz[0:65, 64 * h:64 * (h + 1)],
                            lhsT=vones[:, h, :],
                            rhs=expk[:, 64 * h:64 * (h + 1)],
                            start=True, stop=True)
                    nc.vector.tensor_add(cz_acc, cz_acc, cz)

                    # transpose q_sm into qsmT
                    for t4 in range(4):
                        trp = tr_psum.tile([128, 128], BF, tag="tr")
                        nc.tensor.transpose(
                            trp, qsm[:, 2 * t4:2 * t4 + 2, :], ident)
                        nc.vector.tensor_copy(
                            out=qsmT[:, t4, m * 128:(m + 1) * 128], in_=trp)

                cz_sb = czs_pool.tile([65, 512], BF, tag="cz_sb")
                nc.vector.tensor_copy(out=cz_sb, in_=cz_acc)
                part_b = dram.tile([65, 512], BF, tag=f"part{b}")
                red_b = dram.tile([65, 512], BF, tag=f"red{b}")
                nc.sync.dma_start(out=part_b, in_=cz_sb)
                nc.gpsimd.collective_compute(
                    "AllReduce", mybir.AluOpType.add,
                    replica_groups=[list(range(NCORES))],
                    ins=[part_b.opt()], outs=[red_b.opt()])
                red.append(red_b)

        # wo loaded behind phase-1 traffic; only needed for phase 4
        for h in range(8):
            nc.sync.dma_start(out=wo_sb[:, h, :], in_=wo[64 * h:64 * (h + 1), :])

        with ExitStack() as p45:
            work2 = p45.enter_context(tc.tile_pool(name="work2", bufs=2))
            small2 = p45.enter_context(tc.tile_pool(name="small2", bufs=2))
            ysb_pool = p45.enter_context(tc.tile_pool(name="ysb", bufs=4))
            m_psum = p45.enter_context(tc.tile_pool(name="m_ps", bufs=2, space="PSUM"))
            y_psum = p45.enter_context(tc.tile_pool(name="y_ps", bufs=4, space="PSUM"))

            for b in range(B):
                red_sb = work2.tile([65, 512], BF, tag="red")
                nc.sync.dma_start(out=red_sb, in_=red[b])
                # Z row -> per-partition [128, 4] via 4 tiny transposes
                ztp = m_psum.tile([128, 4, 2], BF, tag="ztp")
                for j in range(4):
                    nc.tensor.transpose(
                        ztp[:, j, 0:1],
                        red_sb[64:65, 128 * j:128 * (j + 1)],
                        ones128[64:65, :])
                rz = small2.tile([128, 4], F32, tag="rz")
                nc.vector.reciprocal(rz, ztp[:, :, 0])

                # M_t = ctx_t @ Wout_t with 1/Z folded in on the copy out
                # head h = 2t + r: ctx_h = red_sb[0:64, 64h:64h+64]
                m_sb = work2.tile([128, 4, D], BF, tag="msb")
                for t in range(4):
                    for cb in range(2):
                        mp = m_psum.tile([128, 512], F32, tag="mp")
                        for r in range(2):
                            h = 2 * t + r
                            nc.tensor.matmul(
                                mp[64 * r:64 * (r + 1), :],
                                lhsT=red_sb[0:64, 64 * h:64 * (h + 1)],
                                rhs=wo_sb[:, h, cb * 512:(cb + 1) * 512],
                                start=True, stop=True)
                        if cb == 0:
                            nc.scalar.activation(
                                out=m_sb[:, t, cb * 512:(cb + 1) * 512],
                                in_=mp, func=COPY, scale=rz[:, t:t + 1])
                        else:
                            nc.vector.tensor_scalar_mul(
                                m_sb[:, t, cb * 512:(cb + 1) * 512], mp,
                                rz[:, t:t + 1])

                # y = sum_t qsmT_t^T @ M_t
                for mi in range(NT_B):
                    for cb in range(2):
                        yp = y_psum.tile([128, 512], F32, tag="yp")
                        for t in range(4):
                            nc.tensor.matmul(
                                yp, lhsT=qsmT[:, t, b * NL + mi * 128:
                                              b * NL + (mi + 1) * 128],
                                rhs=m_sb[:, t, cb * 512:(cb + 1) * 512],
                                start=(t == 0), stop=(t == 3))
                        ysb = ysb_pool.tile([128, 512], BF, tag="ysb")
                        if cb == 0:
                            nc.scalar.activation(out=ysb, in_=yp, func=COPY)
                        else:
                            nc.vector.tensor_copy(out=ysb, in_=yp)
                        nc.sync.dma_start(
                            out=y[b * NL + mi * 128: b * NL + (mi + 1) * 128,
                                  cb * 512:(cb + 1) * 512],
                            in_=ysb)


_COMPILED = None


def _build():
    global _COMPILED
    if _COMPILED is None:
        nc = bacc.Bacc("TRN2", target_bir_lowering=False, debug=False,
                       num_devices=NCORES)
        xT = nc.declare_dram_parameter("xT", [D, SEQ], BF, isOutput=False)
        wq = nc.declare_dram_parameter("wq", [D, 3 * INNER], BF, isOutput=False)
        wo = nc.declare_dram_parameter("wo", [INNER, D], BF, isOutput=False)
        y = nc.declare_dram_parameter("y", [SEQ, D], BF, isOutput=True)
        with tile.TileContext(nc) as tc:
            _body(tc, xT, wq, wo, y)
        nc.compile()
        _COMPILED = nc
    return _COMPILED


def _make_in_maps(x, W_qkv, W_out):
    wq_bf = np.ascontiguousarray(W_qkv).astype(bf16)
    wo_bf = np.ascontiguousarray(W_out).astype(bf16)
    in_maps = []
    for c in range(NCORES):
        rows = slice(c * NL, (c + 1) * NL)
        xs = np.concatenate([x[0, rows], x[1, rows]], axis=0)  # [2048, 1024]
        xT_bf = np.ascontiguousarray(xs.T).astype(bf16)        # [1024, 2048]
        in_maps.append({"xT": xT_bf, "wq": wq_bf, "wo": wo_bf})
    return in_maps


def _run(x, W_qkv, W_out, b_out, trace=False, **spmd_kwargs):
    nc = _build()
    in_maps = _make_in_maps(x, W_qkv, W_out)
    res = run_bass_kernel_spmd(nc, in_maps, list(range(NCORES)),
                               trace=trace, **spmd_kwargs)
    out = np.empty((B, N, D), np.float32)
    for c in range(NCORES):
        yc = np.asarray(res.results[c]["y"]).astype(np.float32)
        rows = slice(c * NL, (c + 1) * NL)
        out[0, rows] = yc[:NL]
        out[1, rows] = yc[NL:]
    out += np.asarray(b_out, np.float32)[None, None, :]
    return out, res


def kernel(x, W_qkv, W_out, b_out):
    x = np.asarray(x, np.float32)
    out, _ = _run(x, np.asarray(W_qkv, np.float32),
                  np.asarray(W_out, np.float32),
                  np.asarray(b_out, np.float32))
    return out
